# revision 1
# baseline (speedup 1.0000x reference)
"""Multi-head self-attention (N=4, T=2048, D=1024, H=16) on 8 TRN2 NeuronCores.

Sharding: core c -> (batch n = c//2, head-group g = c%2 of 8 heads).

Mask compaction: the reference masks ~half the QUERY tokens (mask==0) with
-1e20, which makes softmax exactly uniform for those rows -> their context is
mean(V), identical for every masked row of a batch. kernel() therefore gathers
the unmasked query rows on the host (index bookkeeping only), pads to a common
TQ (multiple of 128, >= max_count+1 so at least one all-zero pad row exists),
and the device computes attention only for TQ compacted queries. A zero Q row
gives S==0 -> exp==1 -> exactly the uniform softmax, so pad row c_n doubles as
the shared masked-row output. Host-side scatter puts rows back (gather/scatter
is data marshaling of the same magnitude as the baseline's shard/assemble).

Device kernel per core (all compute on device, bf16 operands):
  - X^T and Xg^T via PE transpose; K^T/V from full X, Q^T from gathered Xg
    with pad columns zeroed via a padmask multiply fused into bias-add.
  - S^T = K^T.T @ Q^T with the PE in 64x128 row-tiled mode: two key blocks
    of the same head stream concurrently on array tiles (0,0)/(64,0), fed by
    half-swapped K^T/Q^T mirror copies; one full-TQ exp per (head, j) on
    ScalarE PSUM->SBUF bf16.
  - ctx^T (+Z row) = [V|1].T @ P^T in 512-wide accumulation chains, the
    128-token contraction split across the same two 64-row PE tiles (so the
    whole attention loop stays in one tiling mode -- mode-switch drains are
    expensive) with a DVE merge of the partials; chains interleave with the
    next head's S stream so ScalarE never starves; normalize with 1/Z.
  - pair AllGather of ctx^T (bf16, two halves overlapping attention), then
    the 512-column slice of the output projection.
"""

from contextlib import ExitStack

import numpy as np

import concourse.bass as bass
import concourse.mybir as mybir
import concourse.tile as tile
from concourse import bacc
from concourse.bass_utils import run_bass_kernel_spmd
from concourse.masks import make_identity

N, T, D, H, DH = 4, 2048, 1024, 16, 64
N_CORES = 8
G = 512            # per-core projection width (8 heads x 64)
HPC = 8            # heads per core
SCALE = 1.0 / 8.0  # 1/sqrt(DH)

f32 = mybir.dt.float32
f32r = mybir.dt.float32r
bf16 = mybir.dt.bfloat16
i32 = mybir.dt.int32

COMPUTE_DT = "bf16"  # {"f32r", "bf16"} dtype for projection/S matmul operands

# global din-block order produced by the two half-AllGathers:
# half 0 carries heads 0-3 (blocks 0,1) + peer heads 8-11 (blocks 4,5)
CC_PERM = [[0, 1, 4, 5], [2, 3, 6, 7]]


def _chunks(total, w):
    out = []
    c0 = 0
    while c0 < total:
        cw = min(w, total - c0)
        out.append((c0, cw))
        c0 += cw
    return out


def build_nc(TQ, compute_dt: str = COMPUTE_DT, single_core=False,
             reps: int = 0, ablate: str = "", rowtile=True,
             ctxrt=True, tailsplit=True, ctxdr=False,
             kdr=0, pace=0, csb=2) -> bacc.Bacc:
    # kdr=3 (fp8-DR ctx on 3 heads) passes the gate at 1.78e-2 but measured
    # 60-77us SLOWER in both A/B orders: the shorter DR ctx chains break the
    # S/ctx interleave pacing and dual-V evictions grow phase 1.
    # NOTE: ctxdr (fp8 DoubleRow ctx) is ~90us faster but fails the 2e-2
    # correctness gate (rel err 2.85e-2: fp8 quantization of P and V scales
    # with the signal in the weighted sum, ~3.6% each) -- kept for reference
    TQ = int(TQ)
    reps = int(reps)
    single_core = single_core in (True, "True", "1", "true")
    rowtile = rowtile in (True, "True", "1", "true")
    ctxrt = ctxrt in (True, "True", "1", "true")
    tailsplit = tailsplit in (True, "True", "1", "true")
    ctxdr = ctxdr in (True, "True", "1", "true")
    kdr = int(kdr)
    pace = int(pace)
    csb = int(csb)
    cdt = f32r if compute_dt == "f32r" else bf16
    # mirrors + full-width slabs assume a compacted TQ; degenerate
    # nearly-unmasked inputs fall back to the plain S path
    rowtile = rowtile and cdt == bf16 and TQ <= 1536
    ctxrt = ctxrt and rowtile
    # fp8 DoubleRow ctx: 256-wide virtual contraction, PE stays in 64-row
    # tiled mode; Ko strides (one key block) must be 16B-aligned. Full-fp8
    # (ctxdr) fails the 2e-2 gate; kdr applies it to only the first kdr
    # heads per core -- error adds per-head in quadrature, so kdr=3 stays
    # inside the gate while recovering ~3/8 of the DoubleRow win.
    ctxdr = ctxdr and ctxrt and TQ % 16 == 0
    if not (ctxrt and TQ % 16 == 0):
        kdr = 0
    if ctxdr:
        kdr = HPC
    fp8 = mybir.dt.float8e4
    VW = 65                        # per-head V section width, bf16 copy
    V8W = 72                       # fp8 copy section width (16B-aligned)
    # softmax is shift-invariant; shifting the exponent keeps exp() well
    # inside fp8e4m3 range (max 448) -- the e^-3 factor cancels in 1/Z
    EXPB_VAL = -3.0 if kdr > 0 else 0.0

    nc = bacc.Bacc(
        "TRN2", target_bir_lowering=False, debug=False, num_devices=N_CORES
    )
    x_d = nc.dram_tensor("x", [T, D], f32, kind="ExternalInput").ap()
    xg_d = nc.dram_tensor("xg", [TQ, D], f32, kind="ExternalInput").ap()
    pm_d = nc.dram_tensor("pm", [TQ], f32, kind="ExternalInput").ap()
    wq_d = nc.dram_tensor("Wq", [G, D], f32, kind="ExternalInput").ap()
    wk_d = nc.dram_tensor("Wk", [G, D], f32, kind="ExternalInput").ap()
    wv_d = nc.dram_tensor("Wv", [G, D], f32, kind="ExternalInput").ap()
    wo_d = nc.dram_tensor("Wo", [G, D], f32, kind="ExternalInput").ap()
    bq_d = nc.dram_tensor("bq", [G], f32, kind="ExternalInput").ap()
    bk_d = nc.dram_tensor("bk", [G], f32, kind="ExternalInput").ap()
    bv_d = nc.dram_tensor("bv", [G], f32, kind="ExternalInput").ap()
    bo_d = nc.dram_tensor("bo", [G], f32, kind="ExternalInput").ap()
    out_d = nc.dram_tensor("out", [TQ, G], f32, kind="ExternalOutput").ap()

    TB = T // 128     # 16 key token blocks
    DB = D // 128     # 8 feature blocks
    GB = G // 128     # 4 projected blocks
    TQB = TQ // 128   # compacted query blocks
    # S/exp works on full-width [128, SW] PSUM tiles (one ACT per (h, j, sc));
    # ctx accumulates in 512-wide chains so PSUM fits: 2*banks(SW)+2 <= 8.
    SW = TQ if TQ <= 1536 else 1024
    SCH = _chunks(TQ, SW)     # S/exp chunk granularity
    CH = _chunks(TQ, 512)     # ctx chunk granularity

    with tile.TileContext(nc) as tc, ExitStack() as outer_ctx:
        if reps:
            outer_ctx.enter_context(tc.For_i(0, reps, 1))
        ctx = outer_ctx.enter_context(ExitStack())
        const = ctx.enter_context(tc.tile_pool(name="const", bufs=1))
        identity = None
        if cdt != bf16:
            identity = const.tile([128, 128], f32)
            make_identity(nc, identity)
        identity_b = const.tile([128, 128], bf16, tag="idb")
        make_identity(nc, identity_b)
        bqk = const.tile([128, 2 * GB], f32, tag="bqk")
        bq_c, bk_c = bqk[:, 0:GB], bqk[:, GB:2 * GB]
        bob = const.tile([128, G], f32, tag="bvo")
        expb_t = const.tile([128, 1], f32, tag="expb")
        nc.gpsimd.memset(expb_t[:], EXPB_VAL)
        EXPB = expb_t[:]

        qpool = ctx.enter_context(tc.tile_pool(name="qpool", bufs=1))
        q_t = [qpool.tile([128, TQ], cdt, tag=f"q{i}", name=f"q{i}")
               for i in range(GB)]
        k_t = [qpool.tile([128, T], cdt, tag=f"k{i}", name=f"k{i}")
               for i in range(GB)]
        if rowtile:
            # half-swapped mirrors so both PE row-tiles (partitions 0-63 and
            # 64-127) can stream the SAME head's K^T/Q^T concurrently
            q_m = [qpool.tile([128, TQ], cdt, tag=f"qm{i}", name=f"qm{i}")
                   for i in range(GB)]
            k_m = [qpool.tile([128, T], cdt, tag=f"km{i}", name=f"km{i}")
                   for i in range(GB)]
        v_all = qpool.tile([128, TB * HPC * VW], bf16, tag="v_all")
        v_t = [v_all[:, i * HPC * VW:(i + 1) * HPC * VW] for i in range(TB)]
        if kdr > 0:
            v8_all = qpool.tile([128, TB * HPC * V8W], fp8, tag="v8_all")
            v8_t = [v8_all[:, i * HPC * V8W:(i + 1) * HPC * V8W]
                    for i in range(TB)]

        dram = ctx.enter_context(tc.tile_pool(name="dram", bufs=1,
                                              space="DRAM"))
        cc_in = dram.tile([G, TQ], bf16)
        cc_out = [dram.tile([G, TQ], bf16, name=f"cc_out{c}",
                            tag=f"cc_out{c}") for c in range(2)]

        maskp = ctx.enter_context(ExitStack())
        mkpool = maskp.enter_context(tc.tile_pool(name="maskp", bufs=1))
        pmb = mkpool.tile([128, TQ], f32, tag="pmb")
        bvb = mkpool.tile([128, G], f32, tag="bvb")

        # ---- padmask + biases ----
        with tc.tile_pool(name="mload", bufs=1) as mp:
            pm_r = mp.tile([1, TQ], f32)
            nc.sync.dma_start(pm_r[:], pm_d[None, :])
            nc.gpsimd.partition_broadcast(pmb[:], pm_r[:])
            nc.sync.dma_start(bq_c[:], bq_d.rearrange("(j p) -> p j", p=128))
            nc.sync.dma_start(bk_c[:], bk_d.rearrange("(j p) -> p j", p=128))
            bv_r = mp.tile([1, G], f32, tag="bvr")
            nc.sync.dma_start(bv_r[:], bv_d[None, :])
            nc.gpsimd.partition_broadcast(bvb[:], bv_r[:])
            bo_r = mp.tile([1, G], f32, tag="bor")
            nc.sync.dma_start(bo_r[:], bo_d[None, :])
            nc.gpsimd.partition_broadcast(bob[:], bo_r[:])

        # ---- phase 1: X^T, Xg^T, W^T, projections ----
        p1 = ctx.enter_context(ExitStack())
        xtp = p1.enter_context(tc.tile_pool(name="xt", bufs=1))
        sp = p1.enter_context(tc.tile_pool(name="stage", bufs=3))
        wtp = p1.enter_context(tc.tile_pool(
            name="wt", bufs=16 if cdt == bf16 else 8))
        pp = p1.enter_context(tc.tile_pool(name="pp", bufs=5, space="PSUM"))
        xt = [xtp.tile([128, T], cdt, tag=f"xt{d}", name=f"xt{d}")
              for d in range(DB)]
        xgt = [xtp.tile([128, TQ], cdt, tag=f"xg{d}", name=f"xg{d}")
               for d in range(DB)]

        def stage_transpose(src_d, dst, i):
            # load f32 rows, cast to bf16 on DVE (idle in phase 1), then
            # transpose via the DMA xbar -- no PE time at all
            xs = sp.tile([128, D], f32, tag="stage", name="stage")
            if i < 2:
                # split the pipe-filling loads so the first transpose can
                # start as early as possible
                for ii in range(4):
                    nc.sync.dma_start(
                        xs[ii * 32:(ii + 1) * 32, :],
                        src_d[i * 128 + ii * 32:i * 128 + (ii + 1) * 32, :],
                    )
            else:
                nc.sync.dma_start(xs[:], src_d[i * 128:(i + 1) * 128, :])
            if cdt == bf16:
                xb = sp.tile([128, D], bf16, tag="stageb", name="stageb")
                nc.vector.tensor_copy(xb[:], xs[:])
                for d in range(DB):
                    ps = pp.tile([128, 512], bf16, tag="pp", name="pp")
                    nc.tensor.transpose(
                        ps[:, 0:128], xb[:, d * 128:(d + 1) * 128],
                        identity_b[:]
                    )
                    nc.any.tensor_copy(dst[d][:, i * 128:(i + 1) * 128],
                                       ps[:, 0:128])
            else:
                for d in range(DB):
                    ps = pp.tile([128, 512], f32, tag="pp", name="pp")
                    nc.tensor.transpose(
                        ps[:, 0:128], xs[:, d * 128:(d + 1) * 128],
                        identity[:]
                    )
                    nc.any.tensor_copy(dst[d][:, i * 128:(i + 1) * 128],
                                       ps[:, 0:128])

        for i in range(TB):
            stage_transpose(x_d, xt, i)
        for i in range(TQB):
            stage_transpose(xg_d, xgt, i)

        def load_wT(w_dram):
            tiles = [wtp.tile([128, G], cdt, tag="wt", name="wt")
                     for _ in range(DB)]
            for r in range(GB):
                ws = sp.tile([128, D], f32, tag="stage", name="stage")
                nc.sync.dma_start(ws[:], w_dram[r * 128:(r + 1) * 128, :])
                if cdt == bf16:
                    wb = sp.tile([128, D], bf16, tag="stageb", name="stageb")
                    nc.vector.tensor_copy(wb[:], ws[:])
                    for d in range(DB):
                        ps = pp.tile([128, 512], bf16, tag="pp", name="pp")
                        nc.tensor.transpose(
                            ps[:, 0:128], wb[:, d * 128:(d + 1) * 128],
                            identity_b[:]
                        )
                        nc.any.tensor_copy(
                            tiles[d][:, r * 128:(r + 1) * 128], ps[:, 0:128]
                        )
                    continue
                for d in range(DB):
                    ps = pp.tile([128, 512], f32, tag="pp", name="pp")
                    nc.tensor.transpose(
                        ps[:, 0:128], ws[:, d * 128:(d + 1) * 128],
                        identity[:]
                    )
                    nc.any.tensor_copy(
                        tiles[d][:, r * 128:(r + 1) * 128], ps[:, 0:128]
                    )
            return tiles

        # Q^T from gathered queries; pad columns zeroed by the padmask
        # multiply (pad rows of xg are zero, but biases would repopulate them)
        # Q/K projections hold each weight block stationary across all moving
        # chunks (d-major inner loop) so LDWEIGHTS runs once per (b, d)
        wqT = load_wT(wq_d)
        qch = _chunks(TQ, 512)
        for b in range(GB):
            pss = [pp.tile([128, 512], f32, tag="pp", name="pp")
                   for _ in qch]
            for d in range(DB):
                for ci, (c0, cw) in enumerate(qch):
                    nc.tensor.matmul(
                        pss[ci][:, 0:cw],
                        wqT[d][:, b * 128:(b + 1) * 128],
                        xgt[d][:, c0:c0 + cw],
                        start=(d == 0),
                        stop=(d == DB - 1),
                    )
            for ci, (c0, cw) in enumerate(qch):
                nc.vector.scalar_tensor_tensor(
                    q_t[b][:, c0:c0 + cw],
                    pss[ci][:, 0:cw],
                    bq_c[:, b:b + 1],
                    pmb[:, c0:c0 + cw],
                    op0=mybir.AluOpType.add,
                    op1=mybir.AluOpType.mult,
                )
            if rowtile:
                nc.sync.dma_start(q_m[b][0:64, :], q_t[b][64:128, :])
                nc.sync.dma_start(q_m[b][64:128, :], q_t[b][0:64, :])
        wkT = load_wT(wk_d)
        for b in range(GB):
            pss = [pp.tile([128, 512], f32, tag="pp", name="pp")
                   for _ in range(4)]
            for d in range(DB):
                for tch in range(4):
                    nc.tensor.matmul(
                        pss[tch][:],
                        wkT[d][:, b * 128:(b + 1) * 128],
                        xt[d][:, tch * 512:(tch + 1) * 512],
                        start=(d == 0),
                        stop=(d == DB - 1),
                    )
            for tch in range(4):
                nc.vector.tensor_scalar_add(
                    k_t[b][:, tch * 512:(tch + 1) * 512], pss[tch][:],
                    bk_c[:, b:b + 1]
                )
            if rowtile:
                nc.sync.dma_start(k_m[b][0:64, :], k_t[b][64:128, :])
                nc.sync.dma_start(k_m[b][64:128, :], k_t[b][0:64, :])
        # V token-major [t, dout] with a ones column per head
        wvT = load_wT(wv_d)
        for i in range(TB):
            nc.gpsimd.memset(v_t[i][:], 1.0)
            if kdr > 0:
                nc.gpsimd.memset(v8_t[i][:], 1.0)
            ps = pp.tile([128, 512], f32, tag="pp", name="pp")
            for d in range(DB):
                nc.tensor.matmul(
                    ps[:],
                    xt[d][:, i * 128:(i + 1) * 128],
                    wvT[d][:],
                    start=(d == 0),
                    stop=(d == DB - 1),
                )
            for h in range(HPC):
                nc.vector.tensor_tensor(
                    v_t[i][:, h * VW:h * VW + 64],
                    ps[:, h * 64:(h + 1) * 64],
                    bvb[:, h * 64:(h + 1) * 64],
                    op=mybir.AluOpType.add,
                )
                if h < kdr:
                    nc.vector.tensor_tensor(
                        v8_t[i][:, h * V8W:h * V8W + 64],
                        ps[:, h * 64:(h + 1) * 64],
                        bvb[:, h * 64:(h + 1) * 64],
                        op=mybir.AluOpType.add,
                    )

        p1.close()
        maskp.close()

        if "p1only" in ablate:
            with tc.tile_pool(name="dump", bufs=1) as dp:
                zt = dp.tile([128, G], f32)
                nc.vector.tensor_copy(zt[:, 0:G], bob[:, 0:G])
                for i in range(TQB):
                    nc.sync.dma_start(out_d[i * 128:(i + 1) * 128, :], zt[:])
            skip_att = True
        else:
            skip_att = False

        if not skip_att:
            # ---- phase 2: attention; ctx runs one unit behind S/exp ----
            slabp = ctx.enter_context(tc.tile_pool(name="slab", bufs=2))
            zp = ctx.enter_context(tc.tile_pool(name="zbuf", bufs=2))
            csp = ctx.enter_context(tc.tile_pool(name="cstage", bufs=csb))
            spp = ctx.enter_context(tc.tile_pool(name="spsum", bufs=2,
                                                 space="PSUM"))
            att_late = ExitStack()
            cpp = att_late.enter_context(
                tc.tile_pool(name="cpsum", bufs=2, space="PSUM"))
            woTp = att_late.enter_context(tc.tile_pool(name="wot", bufs=8))
            cf0p = att_late.enter_context(tc.tile_pool(name="cf0", bufs=1))
            woT = [woTp.tile([128, G], bf16, tag="wot", name="wot")
                   for _ in range(DB)]
            cf0 = [cf0p.tile([128, TQ], bf16, tag=f"cf0{j}", name=f"cf0{j}")
                   for j in range(GB)]

            slabs = {}

            def emit_s_tile(h, j):
                # one key-block row of S^T for head h: matmuls + exp, full TQ wide
                qk, hb = h // 2, (h % 2) * 64
                slab = slabs[h]
                for c0, cw in SCH:
                    sps = spp.tile([128, SW], f32, tag="sp", name="sp")
                    for q0, qw in _chunks(cw, 512):
                        nc.tensor.matmul(
                            sps[:, q0:q0 + qw],
                            k_t[qk][hb:hb + 64, j * 128:(j + 1) * 128],
                            q_t[qk][hb:hb + 64, c0 + q0:c0 + q0 + qw],
                            start=True,
                            stop=True,
                        )
                    nc.scalar.activation(
                        slab[:, j * TQ + c0:j * TQ + c0 + cw],
                        sps[:, 0:cw],
                        mybir.ActivationFunctionType.Exp,
                        scale=SCALE,
                    )

            def emit_s_tile_pair(h, j0):
                # two key-block rows of S^T for head h computed CONCURRENTLY
                # on the PE's 64-row tiles T0 (partitions 0-63) and T8
                # (64-127), using the half-swapped mirrors for the other half
                qk, even = h // 2, (h % 2) == 0
                j1 = j0 + 1
                slab = slabs[h]
                if even:
                    k0, q0s = k_t[qk][0:64, :], q_t[qk][0:64, :]
                    k1, q1s = k_m[qk][64:128, :], q_m[qk][64:128, :]
                else:
                    k0, q0s = k_m[qk][0:64, :], q_m[qk][0:64, :]
                    k1, q1s = k_t[qk][64:128, :], q_t[qk][64:128, :]
                for c0, cw in SCH:
                    psa = spp.tile([128, SW], f32, tag="sp", name="sp")
                    psb = spp.tile([128, SW], f32, tag="sp", name="sp")
                    for q0, qw in _chunks(cw, 512):
                        nc.tensor.matmul(
                            psa[:, q0:q0 + qw],
                            k0[:, j0 * 128:(j0 + 1) * 128],
                            q0s[:, c0 + q0:c0 + q0 + qw],
                            start=True, stop=True, tile_position=(0, 0),
                        )
                        nc.tensor.matmul(
                            psb[:, q0:q0 + qw],
                            k1[:, j1 * 128:(j1 + 1) * 128],
                            q1s[:, c0 + q0:c0 + q0 + qw],
                            start=True, stop=True, tile_position=(64, 0),
                        )
                    nc.scalar.activation(
                        slab[:, j0 * TQ + c0:j0 * TQ + c0 + cw],
                        psa[:, 0:cw],
                        mybir.ActivationFunctionType.Exp,
                        scale=SCALE,
                        bias=EXPB,
                    )
                    nc.scalar.activation(
                        slab[:, j1 * TQ + c0:j1 * TQ + c0 + cw],
                        psb[:, 0:cw],
                        mybir.ActivationFunctionType.Exp,
                        scale=SCALE,
                        bias=EXPB,
                    )

            def emit_collective(c, cf_tiles):
                if single_core:
                    nc.sync.dma_start(
                        cc_out[c][0:256, :], cc_in[c * 256:(c + 1) * 256, :]
                    )
                    nc.sync.dma_start(
                        cc_out[c][256:512, :], cc_in[c * 256:(c + 1) * 256, :]
                    )
                else:
                    nc.gpsimd.collective_compute(
                        "AllGather",
                        mybir.AluOpType.bypass,
                        replica_groups=[[0, 1], [2, 3], [4, 5], [6, 7]],
                        ins=[cc_in[c * 256:(c + 1) * 256, :].opt()],
                        outs=[cc_out[c][:].opt()],
                    )
                for j in range(GB):
                    nc.sync.dma_start(
                        cf_tiles[j][:], cc_out[c][j * 128:(j + 1) * 128, :]
                    )

            def emit_woT_prep():
                # stage Wo in half-rows to keep the cst/cstb tags at 2KB/1KB
                for r in range(GB):
                    for half in range(2):
                        ws = csp.tile([128, D // 2], f32, tag="cst",
                                      name="wos")
                        nc.sync.dma_start(
                            ws[:],
                            wo_d[r * 128:(r + 1) * 128,
                                 half * (D // 2):(half + 1) * (D // 2)],
                        )
                        wb = csp.tile([128, D // 2], bf16, tag="cstb",
                                      name="wob")
                        nc.vector.tensor_copy(wb[:], ws[:])
                        for dd in range(DB // 2):
                            d = half * (DB // 2) + dd
                            ps = cpp.tile([128, 512], bf16, tag="cp",
                                          name="cp")
                            nc.tensor.transpose(
                                ps[:, 0:128], wb[:, dd * 128:(dd + 1) * 128],
                                identity_b[:]
                            )
                            nc.vector.tensor_copy(
                                woT[d][:, r * 128:(r + 1) * 128],
                                ps[:, 0:128]
                            )

            def ctx_ops(h):
                """Yield fine-grained closures for head h's ctx chains so they
                interleave with the next head's S matmuls on the PE queue.
                With rowtile, the 128-token contraction splits into two
                64-token halves running concurrently on PE tiles (0,0) and
                (64,0) -- both halves already sit on the right partitions --
                and a DVE add merges the partials."""
                slab = slabs[h]

                if "noctx" in ablate:
                    slabs.pop(h)
                    return

                for c0, cw in CH:
                    cps = cpp.tile([65, 512], f32, tag="cp", name="cp")
                    cps_b = (cpp.tile([65, 512], f32, tag="cp", name="cp")
                             if ctxrt else None)

                    def mk_mm(j, c0=c0, cw=cw, cps=cps, cps_b=cps_b):
                        def f():
                            if h < kdr:
                                jp = j  # j is a block-pair index here
                                lhs = v8_all[:, (2 * jp) * HPC * V8W:
                                             (2 * jp + 2) * HPC * V8W
                                             ].rearrange(
                                    "p (k c) -> p k c", k=2
                                )[:, :, h * V8W:h * V8W + 65]
                                rhs = slab[:, (2 * jp) * TQ:
                                           (2 * jp + 2) * TQ].rearrange(
                                    "p (k t) -> p k t", k=2
                                )[:, :, c0:c0 + cw]
                                nc.tensor.matmul(
                                    cps[:, 0:cw], lhs[0:64], rhs[0:64],
                                    start=(jp == 0), stop=(jp == TB // 2 - 1),
                                    perf_mode=mybir.MatmulPerfMode.DoubleRow,
                                    tile_position=(0, 0),
                                )
                                nc.tensor.matmul(
                                    cps_b[:, 0:cw], lhs[64:128], rhs[64:128],
                                    start=(jp == 0), stop=(jp == TB // 2 - 1),
                                    perf_mode=mybir.MatmulPerfMode.DoubleRow,
                                    tile_position=(64, 0),
                                )
                            elif ctxrt:
                                nc.tensor.matmul(
                                    cps[:, 0:cw],
                                    v_t[j][0:64, h * VW:h * VW + 65],
                                    slab[0:64,
                                         j * TQ + c0:j * TQ + c0 + cw],
                                    start=(j == 0),
                                    stop=(j == TB - 1),
                                    tile_position=(0, 0),
                                )
                                nc.tensor.matmul(
                                    cps_b[:, 0:cw],
                                    v_t[j][64:128, h * VW:h * VW + 65],
                                    slab[64:128,
                                         j * TQ + c0:j * TQ + c0 + cw],
                                    start=(j == 0),
                                    stop=(j == TB - 1),
                                    tile_position=(64, 0),
                                )
                            else:
                                nc.tensor.matmul(
                                    cps[:, 0:cw],
                                    v_t[j][:, h * VW:h * VW + 65],
                                    slab[:, j * TQ + c0:j * TQ + c0 + cw],
                                    start=(j == 0),
                                    stop=(j == TB - 1),
                                )
                        return f

                    for j in range(TB // 2 if h < kdr else TB):
                        yield mk_mm(j)

                    def norm(c0=c0, cw=cw, cps=cps, cps_b=cps_b):
                        if ctxrt:
                            # merge the two 64-token partial sums (DVE may
                            # read only one PSUM operand per instruction)
                            csum = csp.tile([65, 512], f32, tag="csum",
                                            name="csum")
                            nc.vector.tensor_copy(csum[:, 0:cw], cps[:, 0:cw])
                            nc.vector.tensor_tensor(
                                csum[:, 0:cw], csum[:, 0:cw], cps_b[:, 0:cw],
                                op=mybir.AluOpType.add,
                            )
                            src = csum
                        else:
                            src = cps
                        # row 64 holds Z = sum_k exp; scale rows 0..63 by 1/Z
                        zrow = zp.tile([128, 512], f32, tag="z", name="z")
                        nc.vector.tensor_copy(zrow[64:65, 0:cw],
                                              src[64:65, 0:cw])
                        nc.sync.dma_start(zrow[0:1, 0:cw], zrow[64:65, 0:cw])
                        nc.vector.reciprocal(zrow[0:1, 0:cw], zrow[0:1, 0:cw])
                        bct = zp.tile([64, 512], f32, tag="bc", name="bc",
                                      bufs=1)
                        nc.gpsimd.partition_broadcast(bct[:, 0:cw],
                                                      zrow[0:1, 0:cw])
                        cst = csp.tile([64, 512], bf16, tag="cst", name="cst")
                        nc.vector.tensor_tensor(
                            cst[:, 0:cw], src[0:64, 0:cw], bct[:, 0:cw],
                            op=mybir.AluOpType.mult
                        )
                        nc.sync.dma_start(
                            cc_in[h * 64:(h + 1) * 64, c0:c0 + cw],
                            cst[:, 0:cw]
                        )
                    yield norm
                slabs.pop(h)

            # per S tile emitted, drain ~ceil(n_ctx_ops / n_s_units) ctx ops
            # of the previous head so ScalarE never starves while PE runs ctx
            n_ctx_ops = len(CH) * (TB + 1)
            n_s_units = TB // 2 if rowtile else TB
            per_tile = pace or (n_ctx_ops + n_s_units - 1) // n_s_units
            pending = None
            for h in range(HPC):
                slabs[h] = slabp.tile([128, TB * TQ],
                                      fp8 if h < kdr else bf16, tag="slab",
                                      name="slab")
                for ju in range(n_s_units):
                    if rowtile:
                        emit_s_tile_pair(h, 2 * ju)
                    else:
                        emit_s_tile(h, ju)
                    if pending is not None:
                        for _ in range(per_tile):
                            op = next(pending, None)
                            if op is None:
                                break
                            op()
                if pending is not None:
                    for op in pending:
                        op()
                if h == 4:
                    # ctx of heads 0-3 fully emitted -> first gather half
                    emit_collective(0, cf0)
                pending = ctx_ops(h)
                if h == 1:
                    emit_woT_prep()
            # ---- final drain + phase 3 ----
            # the cf0 half of the output projection (available since the
            # first gather) interleaves into the last head's ctx drain using
            # the now-idle S PSUM slots; partials (+bias) accumulate in SBUF
            oacc = csp.tile([128, TQB * G], f32, tag="oacc", name="oacc",
                            bufs=1)

            def pass_a(i):
                ps = spp.tile([128, SW], f32, tag="sp", name="sp")
                for j in range(GB):
                    nc.tensor.matmul(
                        ps[:, 0:G],
                        cf0[j][:, i * 128:(i + 1) * 128],
                        woT[CC_PERM[0][j]][:],
                        start=(j == 0),
                        stop=(j == GB - 1),
                    )
                nc.vector.tensor_tensor(
                    oacc[:, i * G:(i + 1) * G], ps[:, 0:G], bob[:],
                    op=mybir.AluOpType.add,
                )

            if tailsplit:
                ia = 0
                nctx = 0
                for op in pending:
                    op()
                    nctx += 1
                    if nctx % 6 == 0 and ia < TQB:
                        pass_a(ia)
                        ia += 1
                while ia < TQB:
                    pass_a(ia)
                    ia += 1
            else:
                for op in pending:
                    op()

            # second gather half + the cf1 half of the projection
            cf1_all = slabp.tile([128, TB * TQ], bf16, tag="slab", name="cf1")
            cf1 = [cf1_all[:, j * TQ:(j + 1) * TQ] for j in range(GB)]
            emit_collective(1, cf1)
            cf = [cf0, cf1]
            for i in range(TQB):
                ps = spp.tile([128, SW], f32, tag="sp", name="sp")
                if tailsplit:
                    for j in range(GB):
                        nc.tensor.matmul(
                            ps[:, 0:G],
                            cf1[j][:, i * 128:(i + 1) * 128],
                            woT[CC_PERM[1][j]][:],
                            start=(j == 0),
                            stop=(j == GB - 1),
                        )
                    os_ = csp.tile([128, G], f32, tag="ostage",
                                   name="ostage")
                    nc.vector.tensor_tensor(
                        os_[:], ps[:, 0:G], oacc[:, i * G:(i + 1) * G],
                        op=mybir.AluOpType.add,
                    )
                else:
                    first = True
                    for c in range(2):
                        for j in range(GB):
                            nc.tensor.matmul(
                                ps[:, 0:G],
                                cf[c][j][:, i * 128:(i + 1) * 128],
                                woT[CC_PERM[c][j]][:],
                                start=first,
                                stop=(c == 1 and j == GB - 1),
                            )
                            first = False
                    os_ = csp.tile([128, G], f32, tag="ostage",
                                   name="ostage")
                    nc.vector.tensor_tensor(os_[:], ps[:, 0:G], bob[:],
                                            op=mybir.AluOpType.add)
                nc.sync.dma_start(out_d[i * 128:(i + 1) * 128, :], os_[:])
            att_late.close()

    nc.compile()
    return nc


def plan_from_mask(mask):
    idx = [np.nonzero(np.asarray(mask[n]) == 1)[0] for n in range(N)]
    cns = [len(i) for i in idx]
    TQ = ((max(cns) + 1 + 127) // 128) * 128
    return {"idx": idx, "cns": cns, "TQ": TQ}


def shard_inputs(plan, query, mask, Wq, bq, Wk, bk, Wv, bv, Wo, bo):
    TQ = plan["TQ"]
    in_maps = []
    for c in range(N_CORES):
        n, g = c // 2, c % 2
        sl = slice(g * G, (g + 1) * G)
        cn = plan["cns"][n]
        xg = np.zeros((TQ, D), np.float32)
        xg[:cn] = np.asarray(query[n], np.float32)[plan["idx"][n]]
        pm = np.zeros((TQ,), np.float32)
        pm[:cn] = 1.0
        in_maps.append(
            {
                "x": np.ascontiguousarray(query[n], dtype=np.float32),
                "xg": xg,
                "pm": pm,
                "Wq": np.ascontiguousarray(Wq[sl], dtype=np.float32),
                "Wk": np.ascontiguousarray(Wk[sl], dtype=np.float32),
                "Wv": np.ascontiguousarray(Wv[sl], dtype=np.float32),
                "Wo": np.ascontiguousarray(Wo[sl], dtype=np.float32),
                "bq": np.ascontiguousarray(bq[sl], dtype=np.float32),
                "bk": np.ascontiguousarray(bk[sl], dtype=np.float32),
                "bv": np.ascontiguousarray(bv[sl], dtype=np.float32),
                "bo": np.ascontiguousarray(bo[sl], dtype=np.float32),
            }
        )
    return in_maps


def gather_outputs(plan, mask, results):
    out = np.empty((N, T, D), np.float32)
    for n in range(N):
        o = np.concatenate(
            [results[2 * n]["out"], results[2 * n + 1]["out"]], axis=1
        )
        cn = plan["cns"][n]
        out[n][plan["idx"][n]] = o[:cn]
        out[n][np.asarray(mask[n]) == 0] = o[cn]
    return out


def kernel(query, mask, Wq, bq, Wk, bk, Wv, bv, Wo, bo):
    plan = plan_from_mask(mask)
    in_maps = shard_inputs(plan, query, mask, Wq, bq, Wk, bk, Wv, bv, Wo, bo)
    nc = build_nc(plan["TQ"])
    res = run_bass_kernel_spmd(nc, in_maps, list(range(N_CORES)))
    return gather_outputs(plan, mask, res.results)

